# revision 1
# baseline (speedup 1.0000x reference)
"""RNN-T joint network kernel for Trainium2 (8 NeuronCores, data-parallel over B).

Computes logits = relu(f @ W1f.T + g @ W1g.T + b1) @ W2.T + b2 over the
(B, T, U, ...) broadcast grid without materializing the concat tensor.

Strategy (per core, one batch element b):
  - Host pre-transposes/casts operands so every matmul operand arrives with
    its contraction dim on partitions (no on-device transposes).
  - pfT[j,t] = W1f @ f.T, pgT[j,u] = W1g @ g.T + b1 computed once (fp32).
  - Grid flattened u-major: g = u*T + t. For each 2048-point span:
      hT[jc][:, :] = relu(pfT[jc][:, t-slice] + pgT_b1[jc][:, u])  (DVE
      tensor_scalar, fused add+max, bf16 out; pg is the per-partition scalar
      so segments break only at u boundaries -> few large instructions).
      Second matmul: W2T chunks stationary on PE, hT streamed, accumulate
      over 4 K-chunks into PSUM [vocab 128, grid 2048] (4 banks).
      Drain: ScalarE Identity activation with per-partition bias = b2 chunk
      (adds b2 for free), casting to bf16 -> SBUF -> 512KB DMA to DRAM.
  - Output lands as outT[vocab, grid] bf16; host casts/transposes back.
"""

import sys

sys.path.insert(0, "/opt/trn_rl_repo")

import numpy as np

from concourse import bacc, bass, tile, mybir
from concourse.bass_utils import run_bass_kernel_spmd

B, T, U = 8, 200, 101
ENC_H, PRED_H, JH, V = 1024, 320, 512, 1024
PRED_P = 384  # PRED_H zero-padded to a multiple of 128
G = U * T  # 20200 grid points per core, u-major: g = u*T + t
SPAN = 2048
NSPAN = (G + SPAN - 1) // SPAN  # 10
GP = NSPAN * SPAN  # 20480 (padded grid)
UPAD = 104  # pgT columns incl. padding for grid tail (u up to 102)
DVE_DRAIN_VCS = (3, 7)  # vocab chunks whose PSUM drain runs on VectorE

F32 = mybir.dt.float32
BF16 = mybir.dt.bfloat16
AF = mybir.ActivationFunctionType
ALU = mybir.AluOpType

_CACHE = {}


def _build_program():
    nc = bacc.Bacc(None, target_bir_lowering=False)

    fT = nc.declare_dram_parameter("fT", [ENC_H, T], F32, isOutput=False)
    gT = nc.declare_dram_parameter("gT", [PRED_P, U], F32, isOutput=False)
    w1fT = nc.declare_dram_parameter("w1fT", [ENC_H, JH], F32, isOutput=False)
    w1gT = nc.declare_dram_parameter("w1gT", [PRED_P, JH], F32, isOutput=False)
    w2T = nc.declare_dram_parameter("w2T", [JH, V], BF16, isOutput=False)
    b1c = nc.declare_dram_parameter("b1c", [128, 4], F32, isOutput=False)
    b2c = nc.declare_dram_parameter("b2c", [128, 8], F32, isOutput=False)
    outT = nc.declare_dram_parameter("outT", [V, GP], BF16, isOutput=True)

    with tile.TileContext(nc) as tc:
        with (
            tc.tile_pool(name="const", bufs=1) as const,
            tc.tile_pool(name="hbuf", bufs=2) as hbuf,
            tc.tile_pool(name="obuf", bufs=4) as obuf,
            tc.tile_pool(name="psum", bufs=2, space="PSUM") as psum,
        ):
            # ---- load inputs (small tensors first; HWDGE ring drains FIFO) ----
            g_sb = const.tile([128, 3, U], F32, tag="g_sb")
            nc.sync.dma_start(g_sb[:], gT[:, :].rearrange("(c p) u -> p c u", p=128))
            w1g_sb = const.tile([128, 3, JH], F32, tag="w1g_sb")
            nc.sync.dma_start(
                w1g_sb[:], w1gT[:, :].rearrange("(c p) j -> p c j", p=128)
            )
            b1_sb = const.tile([128, 4], F32, tag="b1_sb")
            nc.sync.dma_start(b1_sb[:, :], b1c[:, :])
            b2_sb = const.tile([128, 8], F32, tag="b2_sb")
            nc.sync.dma_start(b2_sb[:, :], b2c[:, :])
            # f/W1f stream in 2-chunk pieces so pf matmuls start early
            f_sb = const.tile([128, 8, T], F32, tag="f_sb")
            w1f_sb = const.tile([128, 8, JH], F32, tag="w1f_sb")
            for q in range(4):
                nc.sync.dma_start(
                    f_sb[:, 2 * q : 2 * q + 2, :],
                    fT[256 * q : 256 * (q + 1), :].rearrange(
                        "(c p) t -> p c t", p=128
                    ),
                )
                nc.sync.dma_start(
                    w1f_sb[:, 2 * q : 2 * q + 2, :],
                    w1fT[256 * q : 256 * (q + 1), :].rearrange(
                        "(c p) j -> p c j", p=128
                    ),
                )
            w2_sb = const.tile([128, 4, V], BF16, tag="w2_sb")
            nc.sync.dma_start(w2_sb[:], w2T[:, :].rearrange("(c p) v -> p c v", p=128))

            # ---- first-layer projections (pg first: its inputs land first) ----
            pg_ps = psum.tile([128, 2048], F32, tag="pt")
            for jc in range(4):
                for c in range(3):
                    nc.tensor.matmul(
                        pg_ps[:, jc * 512 : jc * 512 + U],
                        w1g_sb[:, c, jc * 128 : (jc + 1) * 128],
                        g_sb[:, c, :],
                        start=(c == 0),
                        stop=(c == 2),
                    )
            # pgT + b1, padded with zeros for the grid tail (u >= U)
            pg_sb = const.tile([128, 4 * UPAD], F32, tag="pg_sb")
            nc.vector.memset(pg_sb[:, :], 0.0)
            for jc in range(4):
                nc.vector.tensor_scalar(
                    pg_sb[:, jc * UPAD : jc * UPAD + U],
                    pg_ps[:, jc * 512 : jc * 512 + U],
                    b1_sb[:, jc : jc + 1],
                    None,
                    ALU.add,
                )
            # pfT[j, t] accumulated per joint-chunk jc into psum bank jc;
            # hc inner-most pairs with the chunked f/w1f DMAs above
            pf_ps = psum.tile([128, 2048], F32, tag="pt")
            for hc in range(8):
                for jc in range(4):
                    nc.tensor.matmul(
                        pf_ps[:, jc * 512 : jc * 512 + T],
                        w1f_sb[:, hc, jc * 128 : (jc + 1) * 128],
                        f_sb[:, hc, :],
                        start=(hc == 0),
                        stop=(hc == 7),
                    )
            pf_sb = const.tile([128, 4 * T], F32, tag="pf_sb")
            for jc in range(4):
                nc.vector.tensor_copy(
                    pf_sb[:, jc * T : (jc + 1) * T], pf_ps[:, jc * 512 : jc * 512 + T]
                )

            # ---- main loop over grid spans (last span trimmed to the real grid) ----
            for s in range(NSPAN):
                g0 = s * SPAN
                glen = min(SPAN, G - g0)
                # PSUM bank slices covering glen (<=512 each)
                banks = [
                    (b0, min(512, glen - b0)) for b0 in range(0, glen, 512)
                ]
                hts = []
                for jc in range(4):
                    ht = hbuf.tile([128, SPAN], BF16, tag=f"h{jc}")
                    hts.append(ht)
                    g = g0
                    while g < g0 + glen:
                        u, t = g // T, g % T
                        seglen = min(T - t, g0 + glen - g)
                        nc.vector.tensor_scalar(
                            ht[:, g - g0 : g - g0 + seglen],
                            pf_sb[:, jc * T + t : jc * T + t + seglen],
                            pg_sb[:, jc * UPAD + u : jc * UPAD + u + 1],
                            0.0,
                            ALU.add,
                            ALU.max,
                        )
                        g += seglen
                for vc in range(8):
                    pt = psum.tile([128, 2048], F32, tag="pt")
                    for jc in range(4):
                        for bh, (b0, blen) in enumerate(banks):
                            nc.tensor.matmul(
                                pt[:, bh * 512 : bh * 512 + blen],
                                w2_sb[:, jc, vc * 128 : (vc + 1) * 128],
                                hts[jc][:, b0 : b0 + blen],
                                start=(jc == 0),
                                stop=(jc == 3),
                            )
                    ob = obuf.tile([128, SPAN], BF16, tag="ob")
                    if vc in DVE_DRAIN_VCS:
                        # drain on VectorE (fused +b2), balancing ScalarE load
                        nc.vector.tensor_scalar(
                            ob[:, :glen],
                            pt[:, :glen],
                            b2_sb[:, vc : vc + 1],
                            None,
                            ALU.add,
                        )
                    else:
                        nc.scalar.activation(
                            ob[:, :glen],
                            pt[:, :glen],
                            AF.Identity,
                            bias=b2_sb[:, vc : vc + 1],
                            scale=1.0,
                        )
                    nc.sync.dma_start(
                        outT[vc * 128 : (vc + 1) * 128, g0 : g0 + glen], ob[:, :glen]
                    )

    nc.compile()
    return nc


def _get_program():
    if "nc" not in _CACHE:
        _CACHE["nc"] = _build_program()
    return _CACHE["nc"]


def _prep_inputs(f, g, W1, b1, W2, b2):
    bf16 = mybir.dt.np(BF16)
    W1fT = np.ascontiguousarray(W1[:, :ENC_H].T)  # (1024, 512) f32
    W1gT = np.zeros((PRED_P, JH), dtype=np.float32)
    W1gT[:PRED_H] = W1[:, ENC_H:].T  # (384, 512) f32, zero-padded
    W2T = np.ascontiguousarray(W2.T).astype(bf16)  # (512, 1024) bf16
    b1c = np.ascontiguousarray(b1.reshape(4, 128).T).astype(np.float32)
    b2c = np.ascontiguousarray(b2.reshape(8, 128).T).astype(np.float32)
    in_maps = []
    for i in range(B):
        gTp = np.zeros((PRED_P, U), dtype=np.float32)
        gTp[:PRED_H] = g[i].T
        in_maps.append(
            {
                "fT": np.ascontiguousarray(f[i].T).astype(np.float32),
                "gT": gTp,
                "w1fT": W1fT,
                "w1gT": W1gT,
                "w2T": W2T,
                "b1c": b1c,
                "b2c": b2c,
            }
        )
    return in_maps


def run_on_device(f, g, W1, b1, W2, b2, **spmd_kwargs):
    """Runs the kernel; returns (logits, BassKernelResults)."""
    nc = _get_program()
    in_maps = _prep_inputs(f, g, W1, b1, W2, b2)
    res = run_bass_kernel_spmd(nc, in_maps, list(range(B)), **spmd_kwargs)
    out = np.empty((B, T, U, V), dtype=np.float32)
    for i in range(B):
        oT = res.results[i]["outT"][:, :G].astype(np.float32)  # (V, G)
        out[i] = oT.reshape(V, U, T).transpose(2, 1, 0)
    return out, res


def kernel(f, g, W1, b1, W2, b2):
    out, _ = run_on_device(f, g, W1, b1, W2, b2)
    return out

